# revision 41
# baseline (speedup 1.0000x reference)
"""DialogSeqAttnMatch Trainium2 kernel (8-core SPMD, L1-sharded).

Math (reference):
  dlg   = concat(xq, xa) reshaped (B*M, H); M = LQ+LA
  x_proj = relu(xd @ W.T + b);  y_proj = relu(dlg @ W.T + b)
  scores[b,l,k] = x_proj[b,l] . y_proj[k]  masked (causal: ts(k) >= b, padding)
                  + rw0*|b - ts(k)|  (row 0 zeroed)
  out = softmax_k(scores) @ dlg   (row 0 of alpha zeroed -> out[0] = 0)

Key simplifications used here:
  - In the causally valid region ts(k) < b, so |b-ts| = b - ts separates:
    exp(s + rw0*b - rw0*ts) = exp(s) * e^{rw0*b} * e^{-rw0*ts}.  The row
    factor cancels in softmax; the column factor phi_k is folded into the
    value rows on the host.
  - Padding mask: zero the (phi-scaled) value rows AND the appended
    ones-column on the host, so masked tokens drop out of numerator and
    denominator.
  - Causal mask: per (batch, k-chunk) memsets of the exp'd probability
    tile (token timesteps are 64-aligned so the cuts are at 64-row
    boundaries).
  - Output = (P @ [phi*v, phi]) -> divide columns 0:128 by column 128.

Device layout (per core, l-slice of 64 rows for all 32 batches):
  xdT   (128 d, 2048 (b,l))  f32r   scores computed TRANSPOSED: (k, l)
  dlgT  (128 d, 2048 k)      f32r
  x_projT = relu(Wt.T @ xdT + b): (128 h, 2048 (b,l)) f32r
  y_projT = relu(Wt.T @ dlgT + b): (128 h, 2048 k)    f32r
  groups of 4 batches (256 l-cols); k-chunks of 128; per chunk:
    scoresT psum (128k, 256l) = y_projT_chunk^T @ x_projT_group  [fp32r]
    pT = exp(scoresT - S0) bf16 (stacked 4 chunks per ACT instruction;
    the final stack leads with the last chunk whose psA half is dead, so
    the exp window starts at col 128)
    out_psum(A/B) (128 l, 129) += pT_half^T @ dlg_aug_chunk      [bf16]
  normalize: out[:, :128] * recip(out[:, 128]) -> DMA out.

Scheduling notes (what the timeline model rewards):
  - act-table load (1283ns) hoisted to t~0 via a dummy exp.
  - PE pstate warm-up: 4 dummy bf16 matmuls on memset tiles while the
    first DMAs are in flight.
  - group order [0, 1, 3, 4, 5, 6, 7, 2]: g0 (2 chunks) first so the exp
    train starts on a minimal dependency chain; g2 last so only a small
    group trails the final exp.
  - causal-boundary memsets split across Pool (last chunk) and DVE
    (second-to-last) so the tail is not serialized on one engine.
"""
import os
import sys

sys.path.insert(0, "/opt/trn_rl_repo")

import numpy as np
import ml_dtypes

import concourse.bass as bass
import concourse.tile as tile
import concourse.mybir as mybir
from concourse import bacc
from concourse.bass_utils import run_bass_kernel_spmd

F32 = mybir.dt.float32
F32R = mybir.dt.float32r
BF16 = mybir.dt.bfloat16

B, L1, LQ, LA, H = 32, 512, 32, 32, 128
M = LQ + LA              # 64 tokens per timestep
K = B * M                # 2048 flattened history
NCORES = 8
LC = L1 // NCORES        # 64 l-rows per core
S0 = 40.0                # exp shift (scores are >= 0, max ~50)
T0 = 16.0                # phi centering
NG = 8                   # batch groups of 4 (256 l-cols each)
STACK = 4                # k-chunks stacked per PSUM buffer / exp instruction

_NC_CACHE = None


def _chunks_of_group(g):
    return 2 * g + 2


def _build():
    nc = bacc.Bacc("TRN2", target_bir_lowering=False, debug=False)

    # host-packed inputs (DMA'd piece-wise in consumption order):
    #   inp1 = [Wt (128 cols) | bcol (1 col, f32 bits) | dlgT (d x k)] f32r
    #   inp2 = xdT  (d x (b,l)) for this core's l-slice   f32r-rounded
    #   inp3 = dlg_aug, phi-scaled values + ones column, chunk-tiled bf16
    inp1 = nc.dram_tensor("inp1", [H, 129 + K], F32R,
                          kind="ExternalInput").ap()
    inp2 = nc.dram_tensor("inp2", [H, B * LC], F32R, kind="ExternalInput").ap()
    inp3 = nc.dram_tensor("inp3", [128, 16 * 129], BF16, kind="ExternalInput").ap()

    out = nc.dram_tensor("out", [B, LC, H], F32, kind="ExternalOutput").ap()
    out_flat = out.rearrange("b l d -> (b l) d")  # (2048, 128)

    GROUP_ORDER = [0, 1, 3, 4, 5, 6, 7, 2]
    LAST_G = GROUP_ORDER[-1]

    with tile.TileContext(nc) as tc:
        with tc.tile_pool(name="const", bufs=1) as cpool, \
             tc.tile_pool(name="pt", bufs=4) as ptpool, \
             tc.tile_pool(name="osb", bufs=3) as osbpool, \
             tc.tile_pool(name="ps_big", bufs=2, space="PSUM") as psb, \
             tc.tile_pool(name="ps_out", bufs=2, space="PSUM") as pso:

            negs0 = cpool.tile([128, 1], F32)
            nc.vector.memset(negs0[:], -S0)
            # dummy activation at t~0: forces the 1283ns act-table load
            # (auto-inserted before the first InstActivation) to overlap the
            # DMA prologue instead of gating the first exp.
            warm = cpool.tile([128, 1], F32)
            nc.scalar.activation(warm[:], negs0[:],
                                 mybir.ActivationFunctionType.Exp,
                                 bias=negs0[:], scale=1.0)

            # PE pstate warm-up: keep the tensor engine busy on junk matmuls
            # while the first input DMAs land, so the real prologue matmuls
            # run at the ramped clock.
            wdum = cpool.tile([128, 128], BF16)
            rdum = cpool.tile([128, 512], BF16)
            nc.vector.memset(wdum[:].bitcast(F32), 0)
            nc.vector.memset(rdum[:].bitcast(F32), 0)
            for w in range(4):
                pdum = psb.tile([128, 512], F32, tag="proj", bufs=2,
                                name=f"pdum{w}")
                nc.tensor.matmul(pdum[:], wdum[:], rdum[:],
                                 start=True, stop=True)

            i1_sb = cpool.tile([H, 129 + K], F32R)
            wt_sb = i1_sb[:, 0:128]
            bcol_sb = i1_sb[:, 128:129].bitcast(F32)
            dlgT_sb = i1_sb[:, 129:129 + K]
            xdT_sb = cpool.tile([H, B * LC], F32R)
            i3_sb = cpool.tile([128, 16 * 129], BF16)
            dlga_sb = i3_sb[:]  # (128, 2064)

            # DMAs in consumption order (SP issue + HWDGE drain are FIFO;
            # ~1.2us of fixed per-DMA latency, so the head pieces are small
            # for latency and the tail pieces are big for issue-rate).
            nc.sync.dma_start(i1_sb[:, 0:641], inp1[:, 0:641])        # Wt+y0+y1
            nc.sync.dma_start(xdT_sb[:, 0:512], inp2[:, 0:512])       # x g0,g1
            nc.sync.dma_start(xdT_sb[:, 768:1280], inp2[:, 768:1280])  # x g3,g4
            nc.sync.dma_start(i1_sb[:, 641:1153], inp1[:, 641:1153])  # y2,y3
            nc.sync.dma_start(i3_sb[:, 0:1032], inp3[:, 0:1032])      # dlga lo
            nc.sync.dma_start(xdT_sb[:, 1280:2048], inp2[:, 1280:2048])
            nc.sync.dma_start(i1_sb[:, 1153:1665], inp1[:, 1153:1665])  # y4,y5
            nc.sync.dma_start(i1_sb[:, 1665:2177], inp1[:, 1665:2177])  # y6,y7
            nc.sync.dma_start(i3_sb[:, 1032:2064], inp3[:, 1032:2064])  # dlga hi
            nc.sync.dma_start(xdT_sb[:, 512:768], inp2[:, 512:768])   # x g2

            # projections: out[h, col] = relu(sum_d Wt[d, h] * inT[d, col] + b[h])
            # emitted lazily in 256-col pieces; y pieces 0/1 relu on ACT (idle
            # during the prologue), everything else on DVE.
            yproj = cpool.tile([H, K], F32R)
            xproj = cpool.tile([H, B * LC], F32R)

            next_y = [0]
            x_done = set()

            def proj_ps(name):
                return psb.tile([128, 512], F32, tag="proj", bufs=2,
                                name=f"psproj_{name}")

            def emit_relu(dst_slice, ps_slice, on_act):
                if on_act:
                    nc.scalar.activation(dst_slice, ps_slice,
                                         mybir.ActivationFunctionType.Relu,
                                         bias=bcol_sb, scale=1.0)
                else:
                    nc.vector.tensor_scalar(dst_slice, ps_slice, bcol_sb, 0.0,
                                            op0=mybir.AluOpType.add,
                                            op1=mybir.AluOpType.max)

            def need_y(upto, ps_slack=None):
                while next_y[0] * 256 < upto:
                    k = next_y[0]
                    lo = k * 256
                    ps = ps_slack if ps_slack is not None else \
                        proj_ps(f"y{k}")[:, 0:256]
                    ps_slack = None
                    nc.tensor.matmul(ps, wt_sb, dlgT_sb[:, lo:lo + 256],
                                     start=True, stop=True)
                    emit_relu(yproj[:, lo:lo + 256], ps, k < 2)
                    next_y[0] += 1

            def need_x(g, ps_slack=None):
                if g in x_done:
                    return
                x_done.add(g)
                lo = g * 256
                ps = ps_slack if ps_slack is not None else \
                    proj_ps(f"x{g}")[:, 0:256]
                nc.tensor.matmul(ps, wt_sb, xdT_sb[:, lo:lo + 256],
                                 start=True, stop=True)
                emit_relu(xproj[:, lo:lo + 256], ps, False)

            # flat stack list; each stack is (g, [chunk indices by position]).
            # The final stack of a group leads with the LAST chunk: its first
            # 128 pt cols are causally dead (psA skipped), so the exp window
            # starts at col 128 — saves 128 ACT columns per group.
            def stack_sizes(nchunks):
                sizes = []
                rem = nchunks
                while rem > 0:
                    sizes.append(min(STACK, rem))
                    rem -= min(STACK, rem)
                return sizes

            # A group's short (<=2 chunk) closing stack cannot hide the next
            # group's scores latency behind its 505ns exp, so for mid-train
            # groups it is processed SECOND instead of last — the group then
            # closes on a full-size exp.  psA/psB accumulation flags are
            # assigned by processing position (start on first, stop on last),
            # so any chunk processing order is valid.
            group_stacks = {}
            for g in GROUP_ORDER:
                nchunks = _chunks_of_group(g)
                s0 = 0
                stacks = []
                sizes = stack_sizes(nchunks)
                for si, ns in enumerate(sizes):
                    if si == len(sizes) - 1:
                        stacks.append([nchunks - 1] + list(range(s0, nchunks - 1)))
                    else:
                        stacks.append(list(range(s0, s0 + ns)))
                    s0 += ns
                group_stacks[g] = stacks

            flat = []
            first_psA = {}
            last_psA = {}
            last_psB = {}
            for g in GROUP_ORDER:
                nchunks = _chunks_of_group(g)
                seq = [c for st in group_stacks[g] for c in sorted(st)]
                psA_seq = [c for c in seq if c < nchunks - 1]
                first_psA[g] = psA_seq[0]
                last_psA[g] = psA_seq[-1]
                last_psB[g] = seq[-1]
                for si, st in enumerate(group_stacks[g]):
                    flat.append((g, st, si == len(group_stacks[g]) - 1))

            state = {}   # g -> psAB
            tiles = {}   # i -> (ps, pt)

            def emit_scores(i):
                g, chunks, _final = flat[i]
                xg = xproj[:, g * 256:(g + 1) * 256]
                if i in tiles:
                    ps, pt = tiles[i]
                else:
                    ps = psb.tile([128, STACK * 256], F32, tag="scps")
                    pt = ptpool.tile([128, STACK * 256], BF16, tag="pt")
                    tiles[i] = (ps, pt)
                for k, c in enumerate(chunks):
                    nc.tensor.matmul(ps[:, k * 256:(k + 1) * 256],
                                     yproj[:, c * 128:(c + 1) * 128], xg,
                                     start=True, stop=True)

            def emit_process(i):
                g, chunks, final = flat[i]
                ns = len(chunks)
                nchunks = _chunks_of_group(g)
                ps, pt = tiles.pop(i)
                # the stack leading with the group's last chunk has dead cols
                # 0:128 (psA skipped there): start the exp window at col 128.
                off = 128 if chunks[0] == nchunks - 1 else 0
                nc.scalar.activation(pt[:, off:ns * 256], ps[:, off:ns * 256],
                                     mybir.ActivationFunctionType.Exp,
                                     bias=negs0[:], scale=1.0)
                for k, c in enumerate(chunks):
                    blk = pt[:, k * 256:(k + 1) * 256]
                    if c == nchunks - 2:
                        nc.vector.memset(blk[:, 0:64], 0)
                        nc.vector.memset(blk[64:128, 64:128], 0)
                    elif c == nchunks - 1:
                        # cols 0:128 are never read (psA matmul skipped)
                        nc.gpsimd.memset(blk[:, 128:192], 0)
                        nc.gpsimd.memset(blk[64:128, 192:256], 0)
                if min(chunks) == 0:
                    # psA/psB share one PSUM bank: psA's start=True clears the
                    # bank's has_written bits, so psB's first matmul must use
                    # start=False (overwrites the still-clear region).
                    psAB = pso.tile([128, 260], F32, tag="psout", name=f"ps{g}")
                    state[g] = psAB
                psAB = state[g]
                psA = psAB[:, 0:129]
                psB = psAB[:, 130:259]
                # out-MMs in ascending chunk order within the stack; start/
                # stop flags follow the group's chunk PROCESSING order (psA's
                # start=True must be the bank's first write).
                for c in sorted(chunks):
                    k = chunks.index(c)
                    dchunk = dlga_sb[:, c * 129:(c + 1) * 129]
                    blk = pt[:, k * 256:(k + 1) * 256]
                    if c < nchunks - 1:
                        # last chunk's cols 0:128 are causally all-zero: skip
                        nc.tensor.matmul(psA, blk[:, 0:128], dchunk,
                                         start=(c == first_psA[g]),
                                         stop=(c == last_psA[g]))
                    nc.tensor.matmul(psB, blk[:, 128:256], dchunk,
                                     start=False, stop=(c == last_psB[g]))
                if final:
                    emit_normalize(g)

            def emit_normalize(g):
                psAB = state.pop(g)
                psA = psAB[:, 0:129]
                psB = psAB[:, 130:259]
                osb = osbpool.tile([128, 256], F32, tag="osb")
                # both recips first, then both multiplies: saves one DVE
                # serialization hop on the critical tail.
                recips = []
                for half, pshalf in ((0, psA), (1, psB)):
                    recip = osbpool.tile([128, 1], F32, tag="recip")
                    recips.append(recip)
                    if g == 0 and half == 0:
                        nc.vector.reciprocal(recip[64:128, :],
                                             pshalf[64:128, 128:129])
                    else:
                        nc.vector.reciprocal(recip[:], pshalf[:, 128:129])
                for half, pshalf in ((0, psA), (1, psB)):
                    dst = osb[:, half * 128:(half + 1) * 128]
                    recip = recips[half]
                    if g == 0 and half == 0:
                        # batch 0: output is defined as zero (denominator is 0)
                        nc.vector.memset(dst[0:64, :], 0)
                        nc.vector.tensor_scalar_mul(dst[64:128, :],
                                                    pshalf[64:128, 0:128],
                                                    recip[64:128, :])
                    else:
                        nc.vector.tensor_scalar_mul(dst[:], pshalf[:, 0:128],
                                                    recip[:])
                dsl = out_flat[256 * g:256 * (g + 1)].rearrange(
                    "(h p) d -> p h d", h=2)
                osb_src = osb[:].rearrange("p (h d) -> p h d", h=2)
                # Output DMA routing: mid-train groups go through Pool's
                # software DGE (their data-waits would block the SP queue head
                # and delay the final DMA).  The second-to-last group's DMA is
                # deferred to the post-loop ACT queue so its transfer doesn't
                # occupy the DMA lane right when the final DMA (on the
                # then-empty SP queue) needs it.
                if g == LAST_G:
                    nc.sync.dma_start(dsl, osb_src)
                elif g == GROUP_ORDER[-2]:
                    deferred_dma.append((dsl, osb_src))
                else:
                    nc.gpsimd.dma_start(dsl, osb_src)

            # software pipeline: scores(i) ahead of process(i-LOOKAHEAD).
            # process(0) is emitted right after scores(0) so exp(0) sits ahead
            # of the later y-relus in the ACT queue.
            LOOKAHEAD = 2
            done_proc = set()
            deferred_dma = []

            def proc(j):
                if 0 <= j < len(flat) and j not in done_proc:
                    done_proc.add(j)
                    emit_process(j)

            for i in range(len(flat) + LOOKAHEAD):
                if i < len(flat):
                    if i == 0:
                        # stack 0 uses only 512 of its 1024 PSUM cols; put the
                        # two prologue projection pieces (y0, x-g0) in the
                        # slack so they don't serialize on the proj ring.
                        ps0 = psb.tile([128, STACK * 256], F32, tag="scps",
                                       name="ps_s0")
                        pt0 = ptpool.tile([128, STACK * 256], BF16, tag="pt",
                                          name="pt_s0")
                        tiles[0] = (ps0, pt0)
                        need_y(256, ps_slack=ps0[:, 512:768])
                        need_x(GROUP_ORDER[0], ps_slack=ps0[:, 768:1024])
                    # pre-emit projection pieces ~2 stacks ahead: the relu for
                    # the NEXT group must enter the DVE queue before this
                    # group's normalize, or the next group's scores stall.
                    for j in range(i, min(i + 3, len(flat))):
                        gj, chunksj, _ = flat[j]
                        need_y(128 * (max(chunksj) + 1))
                        need_x(gj)
                    emit_scores(i)
                    if i == 0:
                        proc(0)
                proc(i - LOOKAHEAD)

            # second-to-last group's output DMA, issued from the ACT queue
            # after the last exp decode (ACT is idle by then).
            for dsl, osb_src in deferred_dma:
                nc.scalar.dma_start(dsl, osb_src)

    nc.compile()
    return nc


def _get_nc():
    global _NC_CACHE
    if _NC_CACHE is None:
        _NC_CACHE = _build()
    return _NC_CACHE


def _round_f32r(a):
    u = np.ascontiguousarray(a, dtype=np.float32).view(np.uint32)
    r = ((u.astype(np.uint64) + 0x800) & 0xFFFFF000).astype(np.uint32)
    return r.view(np.float32)


LAST_RESULTS = None  # BassKernelResults of the most recent run (for test harness)


def kernel(xd_emb, xq_emb, xa_emb, W, b, recency_weight, xq_mask, xa_mask,
           _trace=False):
    xd_emb = np.asarray(xd_emb, np.float32)
    xq_emb = np.asarray(xq_emb, np.float32)
    xa_emb = np.asarray(xa_emb, np.float32)
    W = np.asarray(W, np.float32)
    b = np.asarray(b, np.float32)
    rw0 = float(np.asarray(recency_weight).reshape(-1)[0])
    pad = np.concatenate([np.asarray(xq_mask), np.asarray(xa_mask)], axis=1).reshape(K)

    dlg = np.concatenate([xq_emb, xa_emb], axis=1).reshape(K, H)
    ts = (np.arange(K) // M).astype(np.float64)
    phi = np.exp(-rw0 * (ts - T0))
    dlg_aug = np.concatenate([dlg.astype(np.float64), np.ones((K, 1))], axis=1)
    dlg_aug *= phi[:, None]
    dlg_aug[pad] = 0.0
    dlga_bf = dlg_aug.astype(ml_dtypes.bfloat16)
    dlga_packed = np.ascontiguousarray(
        dlga_bf.reshape(16, 128, 129).transpose(1, 0, 2).reshape(128, 16 * 129))

    inp1 = np.empty((H, 129 + K), np.float32)
    inp1[:, 0:128] = _round_f32r(W.T)
    inp1[:, 128] = b
    inp1[:, 129:] = _round_f32r(dlg.T)
    inp3 = dlga_packed  # (128, 2064) bf16

    xdT = xd_emb.transpose(2, 0, 1)  # (H, B, L1)
    in_maps = []
    for c in range(NCORES):
        xdT_c = xdT[:, :, c * LC:(c + 1) * LC].reshape(H, B * LC)
        in_maps.append({
            "inp1": inp1,
            "inp2": _round_f32r(xdT_c),
            "inp3": inp3,
        })

    nc = _get_nc()
    try:
        res = run_bass_kernel_spmd(nc, in_maps, list(range(NCORES)),
                                   trace=_trace)
    except ModuleNotFoundError:
        # The axon NTFF-profile hook is absent in this container; if an
        # ambient BASS_TRACE forced the trace path, retry without it.
        os.environ["BASS_NEVER_TRACE"] = "1"
        res = run_bass_kernel_spmd(nc, in_maps, list(range(NCORES)))
    global LAST_RESULTS
    LAST_RESULTS = res
    parts = [res.results[c]["out"] for c in range(NCORES)]
    full = np.concatenate(parts, axis=1)  # (32, 512, 128)
    full[0] = 0.0
    return np.ascontiguousarray(full, dtype=np.float32)


# revision 42
# speedup vs baseline: 1.0147x; 1.0147x over previous
"""DialogSeqAttnMatch Trainium2 kernel (8-core SPMD, L1-sharded).

Math (reference):
  dlg   = concat(xq, xa) reshaped (B*M, H); M = LQ+LA
  x_proj = relu(xd @ W.T + b);  y_proj = relu(dlg @ W.T + b)
  scores[b,l,k] = x_proj[b,l] . y_proj[k]  masked (causal: ts(k) >= b, padding)
                  + rw0*|b - ts(k)|  (row 0 zeroed)
  out = softmax_k(scores) @ dlg   (row 0 of alpha zeroed -> out[0] = 0)

Key simplifications used here:
  - In the causally valid region ts(k) < b, so |b-ts| = b - ts separates:
    exp(s + rw0*b - rw0*ts) = exp(s) * e^{rw0*b} * e^{-rw0*ts}.  The row
    factor cancels in softmax; the column factor phi_k is folded into the
    value rows on the host.
  - Padding mask: zero the (phi-scaled) value rows AND the appended
    ones-column on the host, so masked tokens drop out of numerator and
    denominator.
  - Causal mask: per (batch, k-chunk) memsets of the exp'd probability
    tile (token timesteps are 64-aligned so the cuts are at 64-row
    boundaries).
  - Output = (P @ [phi*v, phi]) -> divide columns 0:128 by column 128.

Device layout (per core, l-slice of 64 rows for all 32 batches):
  xdT   (128 d, 2048 (b,l))  f32r   scores computed TRANSPOSED: (k, l)
  dlgT  (128 d, 2048 k)      f32r
  x_projT = relu(Wt.T @ xdT + b): (128 h, 2048 (b,l)) f32r
  y_projT = relu(Wt.T @ dlgT + b): (128 h, 2048 k)    f32r
  groups of 4 batches (256 l-cols); k-chunks of 128; per chunk:
    scoresT psum (128k, 256l) = y_projT_chunk^T @ x_projT_group  [fp32r]
    pT = exp(scoresT - S0) bf16 (stacked 4 chunks per ACT instruction;
    the final stack leads with the last chunk whose psA half is dead, so
    the exp window starts at col 128)
    out_psum(A/B) (128 l, 129) += pT_half^T @ dlg_aug_chunk      [bf16]
  normalize: out[:, :128] * recip(out[:, 128]) -> DMA out.

Scheduling notes (what the timeline model rewards):
  - act-table load (1283ns) hoisted to t~0 via a dummy exp.
  - PE pstate warm-up: 4 dummy bf16 matmuls on memset tiles while the
    first DMAs are in flight.
  - group order [0, 1, 3, 4, 5, 6, 7, 2]: g0 (2 chunks) first so the exp
    train starts on a minimal dependency chain; g2 last so only a small
    group trails the final exp.
  - causal-boundary memsets split across Pool (last chunk) and DVE
    (second-to-last) so the tail is not serialized on one engine.
"""
import os
import sys

sys.path.insert(0, "/opt/trn_rl_repo")

import numpy as np
import ml_dtypes

import concourse.bass as bass
import concourse.tile as tile
import concourse.mybir as mybir
from concourse import bacc
from concourse.bass_utils import run_bass_kernel_spmd

F32 = mybir.dt.float32
F32R = mybir.dt.float32r
BF16 = mybir.dt.bfloat16

B, L1, LQ, LA, H = 32, 512, 32, 32, 128
M = LQ + LA              # 64 tokens per timestep
K = B * M                # 2048 flattened history
NCORES = 8
LC = L1 // NCORES        # 64 l-rows per core
S0 = 40.0                # exp shift (scores are >= 0, max ~50)
T0 = 16.0                # phi centering
NG = 8                   # batch groups of 4 (256 l-cols each)
STACK = 4                # k-chunks stacked per PSUM buffer / exp instruction

_NC_CACHE = None


def _chunks_of_group(g):
    return 2 * g + 2


def _build():
    nc = bacc.Bacc("TRN2", target_bir_lowering=False, debug=False)

    # host-packed inputs (DMA'd piece-wise in consumption order):
    #   inp1 = [Wt (128 cols) | bcol (1 col, f32 bits) | dlgT (d x k)] f32r
    #   inp2 = xdT  (d x (b,l)) for this core's l-slice   f32r-rounded
    #   inp3 = dlg_aug, phi-scaled values + ones column, chunk-tiled bf16
    inp1 = nc.dram_tensor("inp1", [H, 129 + K], F32R,
                          kind="ExternalInput").ap()
    inp2 = nc.dram_tensor("inp2", [H, B * LC], F32R, kind="ExternalInput").ap()
    inp3 = nc.dram_tensor("inp3", [128, 16 * 129], BF16, kind="ExternalInput").ap()

    out = nc.dram_tensor("out", [B, LC, H], F32, kind="ExternalOutput").ap()
    out_flat = out.rearrange("b l d -> (b l) d")  # (2048, 128)

    GROUP_ORDER = [0, 1, 3, 4, 5, 6, 7, 2]
    LAST_G = GROUP_ORDER[-1]

    with tile.TileContext(nc) as tc:
        with tc.tile_pool(name="const", bufs=1) as cpool, \
             tc.tile_pool(name="pt", bufs=4) as ptpool, \
             tc.tile_pool(name="osb", bufs=3) as osbpool, \
             tc.tile_pool(name="ps_big", bufs=2, space="PSUM") as psb, \
             tc.tile_pool(name="ps_out", bufs=2, space="PSUM") as pso:

            negs0 = cpool.tile([128, 1], F32)
            nc.vector.memset(negs0[:], -S0)
            # dummy activation at t~0: forces the 1283ns act-table load
            # (auto-inserted before the first InstActivation) to overlap the
            # DMA prologue instead of gating the first exp.
            warm = cpool.tile([128, 1], F32)
            nc.scalar.activation(warm[:], negs0[:],
                                 mybir.ActivationFunctionType.Exp,
                                 bias=negs0[:], scale=1.0)

            # PE pstate warm-up: keep the tensor engine busy on junk matmuls
            # while the first input DMAs land, so the real prologue matmuls
            # run at the ramped clock.
            wdum = cpool.tile([128, 128], BF16)
            rdum = cpool.tile([128, 512], BF16)
            nc.vector.memset(wdum[:].bitcast(F32), 0)
            nc.vector.memset(rdum[:].bitcast(F32), 0)
            for w in range(4):
                pdum = psb.tile([128, 512], F32, tag="proj", bufs=2,
                                name=f"pdum{w}")
                nc.tensor.matmul(pdum[:], wdum[:], rdum[:],
                                 start=True, stop=True)

            i1_sb = cpool.tile([H, 129 + K], F32R)
            wt_sb = i1_sb[:, 0:128]
            bcol_sb = i1_sb[:, 128:129].bitcast(F32)
            dlgT_sb = i1_sb[:, 129:129 + K]
            xdT_sb = cpool.tile([H, B * LC], F32R)
            i3_sb = cpool.tile([128, 16 * 129], BF16)
            dlga_sb = i3_sb[:]  # (128, 2064)

            # DMAs in consumption order (SP issue + HWDGE drain are FIFO;
            # ~1.2us of fixed per-DMA latency, so the head pieces are small
            # for latency and the tail pieces are big for issue-rate).
            nc.sync.dma_start(i1_sb[:, 0:385], inp1[:, 0:385])        # Wt+y0
            nc.sync.dma_start(xdT_sb[:, 0:512], inp2[:, 0:512])       # x g0,g1
            nc.sync.dma_start(i1_sb[:, 385:641], inp1[:, 385:641])    # y1
            nc.sync.dma_start(xdT_sb[:, 768:1280], inp2[:, 768:1280])  # x g3,g4
            nc.sync.dma_start(i1_sb[:, 641:1153], inp1[:, 641:1153])  # y2,y3
            nc.sync.dma_start(i3_sb[:, 0:1032], inp3[:, 0:1032])      # dlga lo
            nc.sync.dma_start(xdT_sb[:, 1280:2048], inp2[:, 1280:2048])
            nc.sync.dma_start(i1_sb[:, 1153:1665], inp1[:, 1153:1665])  # y4,y5
            nc.sync.dma_start(i1_sb[:, 1665:2177], inp1[:, 1665:2177])  # y6,y7
            nc.sync.dma_start(i3_sb[:, 1032:2064], inp3[:, 1032:2064])  # dlga hi
            nc.sync.dma_start(xdT_sb[:, 512:768], inp2[:, 512:768])   # x g2

            # projections: out[h, col] = relu(sum_d Wt[d, h] * inT[d, col] + b[h])
            # emitted lazily in 256-col pieces; y pieces 0/1 relu on ACT (idle
            # during the prologue), everything else on DVE.
            yproj = cpool.tile([H, K], F32R)
            xproj = cpool.tile([H, B * LC], F32R)

            next_y = [0]
            x_done = set()

            def proj_ps(name):
                return psb.tile([128, 512], F32, tag="proj", bufs=2,
                                name=f"psproj_{name}")

            def emit_relu(dst_slice, ps_slice, on_act):
                if on_act:
                    nc.scalar.activation(dst_slice, ps_slice,
                                         mybir.ActivationFunctionType.Relu,
                                         bias=bcol_sb, scale=1.0)
                else:
                    nc.vector.tensor_scalar(dst_slice, ps_slice, bcol_sb, 0.0,
                                            op0=mybir.AluOpType.add,
                                            op1=mybir.AluOpType.max)

            def need_y(upto, ps_slack=None):
                while next_y[0] * 256 < upto:
                    k = next_y[0]
                    lo = k * 256
                    ps = ps_slack if ps_slack is not None else \
                        proj_ps(f"y{k}")[:, 0:256]
                    ps_slack = None
                    nc.tensor.matmul(ps, wt_sb, dlgT_sb[:, lo:lo + 256],
                                     start=True, stop=True)
                    emit_relu(yproj[:, lo:lo + 256], ps, k < 2)
                    next_y[0] += 1

            def need_x(g, ps_slack=None):
                if g in x_done:
                    return
                x_done.add(g)
                lo = g * 256
                ps = ps_slack if ps_slack is not None else \
                    proj_ps(f"x{g}")[:, 0:256]
                nc.tensor.matmul(ps, wt_sb, xdT_sb[:, lo:lo + 256],
                                 start=True, stop=True)
                emit_relu(xproj[:, lo:lo + 256], ps, False)

            # flat stack list; each stack is (g, [chunk indices by position]).
            # The final stack of a group leads with the LAST chunk: its first
            # 128 pt cols are causally dead (psA skipped), so the exp window
            # starts at col 128 — saves 128 ACT columns per group.
            def stack_sizes(nchunks):
                sizes = []
                rem = nchunks
                while rem > 0:
                    sizes.append(min(STACK, rem))
                    rem -= min(STACK, rem)
                return sizes

            # A group's short (<=2 chunk) closing stack cannot hide the next
            # group's scores latency behind its 505ns exp, so for mid-train
            # groups it is processed SECOND instead of last — the group then
            # closes on a full-size exp.  psA/psB accumulation flags are
            # assigned by processing position (start on first, stop on last),
            # so any chunk processing order is valid.
            group_stacks = {}
            for g in GROUP_ORDER:
                nchunks = _chunks_of_group(g)
                s0 = 0
                stacks = []
                sizes = stack_sizes(nchunks)
                for si, ns in enumerate(sizes):
                    if si == len(sizes) - 1:
                        stacks.append([nchunks - 1] + list(range(s0, nchunks - 1)))
                    else:
                        stacks.append(list(range(s0, s0 + ns)))
                    s0 += ns
                group_stacks[g] = stacks

            flat = []
            first_psA = {}
            last_psA = {}
            last_psB = {}
            for g in GROUP_ORDER:
                nchunks = _chunks_of_group(g)
                seq = [c for st in group_stacks[g] for c in sorted(st)]
                psA_seq = [c for c in seq if c < nchunks - 1]
                first_psA[g] = psA_seq[0]
                last_psA[g] = psA_seq[-1]
                last_psB[g] = seq[-1]
                for si, st in enumerate(group_stacks[g]):
                    flat.append((g, st, si == len(group_stacks[g]) - 1))

            state = {}   # g -> psAB
            tiles = {}   # i -> (ps, pt)

            def emit_scores(i):
                g, chunks, _final = flat[i]
                xg = xproj[:, g * 256:(g + 1) * 256]
                if i in tiles:
                    ps, pt = tiles[i]
                else:
                    ps = psb.tile([128, STACK * 256], F32, tag="scps")
                    pt = ptpool.tile([128, STACK * 256], BF16, tag="pt")
                    tiles[i] = (ps, pt)
                for k, c in enumerate(chunks):
                    nc.tensor.matmul(ps[:, k * 256:(k + 1) * 256],
                                     yproj[:, c * 128:(c + 1) * 128], xg,
                                     start=True, stop=True)

            def emit_process(i):
                g, chunks, final = flat[i]
                ns = len(chunks)
                nchunks = _chunks_of_group(g)
                ps, pt = tiles.pop(i)
                # the stack leading with the group's last chunk has dead cols
                # 0:128 (psA skipped there): start the exp window at col 128.
                off = 128 if chunks[0] == nchunks - 1 else 0
                nc.scalar.activation(pt[:, off:ns * 256], ps[:, off:ns * 256],
                                     mybir.ActivationFunctionType.Exp,
                                     bias=negs0[:], scale=1.0)
                for k, c in enumerate(chunks):
                    blk = pt[:, k * 256:(k + 1) * 256]
                    if c == nchunks - 2:
                        nc.vector.memset(blk[:, 0:64], 0)
                        nc.vector.memset(blk[64:128, 64:128], 0)
                    elif c == nchunks - 1:
                        # cols 0:128 are never read (psA matmul skipped)
                        nc.gpsimd.memset(blk[:, 128:192], 0)
                        nc.gpsimd.memset(blk[64:128, 192:256], 0)
                if min(chunks) == 0:
                    # psA/psB share one PSUM bank: psA's start=True clears the
                    # bank's has_written bits, so psB's first matmul must use
                    # start=False (overwrites the still-clear region).
                    psAB = pso.tile([128, 260], F32, tag="psout", name=f"ps{g}")
                    state[g] = psAB
                psAB = state[g]
                psA = psAB[:, 0:129]
                psB = psAB[:, 130:259]
                # out-MMs in ascending chunk order within the stack; start/
                # stop flags follow the group's chunk PROCESSING order (psA's
                # start=True must be the bank's first write).
                for c in sorted(chunks):
                    k = chunks.index(c)
                    dchunk = dlga_sb[:, c * 129:(c + 1) * 129]
                    blk = pt[:, k * 256:(k + 1) * 256]
                    if c < nchunks - 1:
                        # last chunk's cols 0:128 are causally all-zero: skip
                        nc.tensor.matmul(psA, blk[:, 0:128], dchunk,
                                         start=(c == first_psA[g]),
                                         stop=(c == last_psA[g]))
                    nc.tensor.matmul(psB, blk[:, 128:256], dchunk,
                                     start=False, stop=(c == last_psB[g]))
                if final:
                    emit_normalize(g)

            def emit_normalize(g):
                psAB = state.pop(g)
                psA = psAB[:, 0:129]
                psB = psAB[:, 130:259]
                osb = osbpool.tile([128, 256], F32, tag="osb")
                # both recips first, then both multiplies: saves one DVE
                # serialization hop on the critical tail.
                recips = []
                for half, pshalf in ((0, psA), (1, psB)):
                    recip = osbpool.tile([128, 1], F32, tag="recip")
                    recips.append(recip)
                    if g == 0 and half == 0:
                        nc.vector.reciprocal(recip[64:128, :],
                                             pshalf[64:128, 128:129])
                    else:
                        nc.vector.reciprocal(recip[:], pshalf[:, 128:129])
                for half, pshalf in ((0, psA), (1, psB)):
                    dst = osb[:, half * 128:(half + 1) * 128]
                    recip = recips[half]
                    if g == 0 and half == 0:
                        # batch 0: output is defined as zero (denominator is 0)
                        nc.vector.memset(dst[0:64, :], 0)
                        nc.vector.tensor_scalar_mul(dst[64:128, :],
                                                    pshalf[64:128, 0:128],
                                                    recip[64:128, :])
                    else:
                        nc.vector.tensor_scalar_mul(dst[:], pshalf[:, 0:128],
                                                    recip[:])
                dsl = out_flat[256 * g:256 * (g + 1)].rearrange(
                    "(h p) d -> p h d", h=2)
                osb_src = osb[:].rearrange("p (h d) -> p h d", h=2)
                # Output DMA routing: mid-train groups go through Pool's
                # software DGE (their data-waits would block the SP queue head
                # and delay the final DMA).  The second-to-last group's DMA is
                # deferred to the post-loop ACT queue so its transfer doesn't
                # occupy the DMA lane right when the final DMA (on the
                # then-empty SP queue) needs it.
                if g == LAST_G:
                    nc.sync.dma_start(dsl, osb_src)
                elif g == GROUP_ORDER[-2]:
                    deferred_dma.append((dsl, osb_src))
                else:
                    nc.gpsimd.dma_start(dsl, osb_src)

            # software pipeline: scores(i) ahead of process(i-LOOKAHEAD).
            # process(0) is emitted right after scores(0) so exp(0) sits ahead
            # of the later y-relus in the ACT queue.
            LOOKAHEAD = 2
            done_proc = set()
            deferred_dma = []

            def proc(j):
                if 0 <= j < len(flat) and j not in done_proc:
                    done_proc.add(j)
                    emit_process(j)

            for i in range(len(flat) + LOOKAHEAD):
                if i < len(flat):
                    if i == 0:
                        # stack 0 uses only 512 of its 1024 PSUM cols; put the
                        # two prologue projection pieces (y0, x-g0) in the
                        # slack so they don't serialize on the proj ring.
                        ps0 = psb.tile([128, STACK * 256], F32, tag="scps",
                                       name="ps_s0")
                        pt0 = ptpool.tile([128, STACK * 256], BF16, tag="pt",
                                          name="pt_s0")
                        tiles[0] = (ps0, pt0)
                        need_y(256, ps_slack=ps0[:, 512:768])
                        need_x(GROUP_ORDER[0], ps_slack=ps0[:, 768:1024])
                    # pre-emit projection pieces ~2 stacks ahead: the relu for
                    # the NEXT group must enter the DVE queue before this
                    # group's normalize, or the next group's scores stall.
                    for j in range(i, min(i + 3, len(flat))):
                        gj, chunksj, _ = flat[j]
                        need_y(128 * (max(chunksj) + 1))
                        need_x(gj)
                    emit_scores(i)
                    if i == 0:
                        proc(0)
                proc(i - LOOKAHEAD)

            # second-to-last group's output DMA, issued from the ACT queue
            # after the last exp decode (ACT is idle by then).
            for dsl, osb_src in deferred_dma:
                nc.scalar.dma_start(dsl, osb_src)

    nc.compile()
    return nc


def _get_nc():
    global _NC_CACHE
    if _NC_CACHE is None:
        _NC_CACHE = _build()
    return _NC_CACHE


def _round_f32r(a):
    u = np.ascontiguousarray(a, dtype=np.float32).view(np.uint32)
    r = ((u.astype(np.uint64) + 0x800) & 0xFFFFF000).astype(np.uint32)
    return r.view(np.float32)


LAST_RESULTS = None  # BassKernelResults of the most recent run (for test harness)


def kernel(xd_emb, xq_emb, xa_emb, W, b, recency_weight, xq_mask, xa_mask,
           _trace=False):
    xd_emb = np.asarray(xd_emb, np.float32)
    xq_emb = np.asarray(xq_emb, np.float32)
    xa_emb = np.asarray(xa_emb, np.float32)
    W = np.asarray(W, np.float32)
    b = np.asarray(b, np.float32)
    rw0 = float(np.asarray(recency_weight).reshape(-1)[0])
    pad = np.concatenate([np.asarray(xq_mask), np.asarray(xa_mask)], axis=1).reshape(K)

    dlg = np.concatenate([xq_emb, xa_emb], axis=1).reshape(K, H)
    ts = (np.arange(K) // M).astype(np.float64)
    phi = np.exp(-rw0 * (ts - T0))
    dlg_aug = np.concatenate([dlg.astype(np.float64), np.ones((K, 1))], axis=1)
    dlg_aug *= phi[:, None]
    dlg_aug[pad] = 0.0
    dlga_bf = dlg_aug.astype(ml_dtypes.bfloat16)
    dlga_packed = np.ascontiguousarray(
        dlga_bf.reshape(16, 128, 129).transpose(1, 0, 2).reshape(128, 16 * 129))

    inp1 = np.empty((H, 129 + K), np.float32)
    inp1[:, 0:128] = _round_f32r(W.T)
    inp1[:, 128] = b
    inp1[:, 129:] = _round_f32r(dlg.T)
    inp3 = dlga_packed  # (128, 2064) bf16

    xdT = xd_emb.transpose(2, 0, 1)  # (H, B, L1)
    in_maps = []
    for c in range(NCORES):
        xdT_c = xdT[:, :, c * LC:(c + 1) * LC].reshape(H, B * LC)
        in_maps.append({
            "inp1": inp1,
            "inp2": _round_f32r(xdT_c),
            "inp3": inp3,
        })

    nc = _get_nc()
    try:
        res = run_bass_kernel_spmd(nc, in_maps, list(range(NCORES)),
                                   trace=_trace)
    except ModuleNotFoundError:
        # The axon NTFF-profile hook is absent in this container; if an
        # ambient BASS_TRACE forced the trace path, retry without it.
        os.environ["BASS_NEVER_TRACE"] = "1"
        res = run_bass_kernel_spmd(nc, in_maps, list(range(NCORES)))
    global LAST_RESULTS
    LAST_RESULTS = res
    parts = [res.results[c]["out"] for c in range(NCORES)]
    full = np.concatenate(parts, axis=1)  # (32, 512, 128)
    full[0] = 0.0
    return np.ascontiguousarray(full, dtype=np.float32)


# revision 43
# speedup vs baseline: 1.0364x; 1.0213x over previous
"""DialogSeqAttnMatch Trainium2 kernel (8-core SPMD, L1-sharded).

Math (reference):
  dlg   = concat(xq, xa) reshaped (B*M, H); M = LQ+LA
  x_proj = relu(xd @ W.T + b);  y_proj = relu(dlg @ W.T + b)
  scores[b,l,k] = x_proj[b,l] . y_proj[k]  masked (causal: ts(k) >= b, padding)
                  + rw0*|b - ts(k)|  (row 0 zeroed)
  out = softmax_k(scores) @ dlg   (row 0 of alpha zeroed -> out[0] = 0)

Key simplifications used here:
  - In the causally valid region ts(k) < b, so |b-ts| = b - ts separates:
    exp(s + rw0*b - rw0*ts) = exp(s) * e^{rw0*b} * e^{-rw0*ts}.  The row
    factor cancels in softmax; the column factor phi_k is folded into the
    value rows on the host.
  - Padding mask: zero the (phi-scaled) value rows AND the appended
    ones-column on the host, so masked tokens drop out of numerator and
    denominator.
  - Causal mask: per (batch, k-chunk) memsets of the exp'd probability
    tile (token timesteps are 64-aligned so the cuts are at 64-row
    boundaries).
  - Output = (P @ [phi*v, phi]) -> divide columns 0:128 by column 128.

Device layout (per core, l-slice of 64 rows for all 32 batches):
  xdT   (128 d, 2048 (b,l))  f32r   scores computed TRANSPOSED: (k, l)
  dlgT  (128 d, 2048 k)      f32r
  x_projT = relu(Wt.T @ xdT + b): (128 h, 2048 (b,l)) f32r
  y_projT = relu(Wt.T @ dlgT + b): (128 h, 2048 k)    f32r
  groups of 4 batches (256 l-cols); k-chunks of 128; per chunk:
    scoresT psum (128k, 256l) = y_projT_chunk^T @ x_projT_group  [fp32r]
    pT = exp(scoresT - S0) bf16 (stacked 4 chunks per ACT instruction;
    the final stack leads with the last chunk whose psA half is dead, so
    the exp window starts at col 128)
    out_psum(A/B) (128 l, 129) += pT_half^T @ dlg_aug_chunk      [bf16]
  normalize: out[:, :128] * recip(out[:, 128]) -> DMA out.

Scheduling notes (what the timeline model rewards):
  - act-table load (1283ns) hoisted to t~0 via a dummy exp.
  - PE pstate warm-up: 4 dummy bf16 matmuls on memset tiles while the
    first DMAs are in flight.
  - group order [0, 1, 3, 4, 5, 6, 7, 2]: g0 (2 chunks) first so the exp
    train starts on a minimal dependency chain; g2 last so only a small
    group trails the final exp.
  - causal-boundary memsets split across Pool (last chunk) and DVE
    (second-to-last) so the tail is not serialized on one engine.
"""
import os
import sys

sys.path.insert(0, "/opt/trn_rl_repo")

import numpy as np
import ml_dtypes

import concourse.bass as bass
import concourse.tile as tile
import concourse.mybir as mybir
from concourse import bacc
from concourse.bass_utils import run_bass_kernel_spmd

F32 = mybir.dt.float32
F32R = mybir.dt.float32r
BF16 = mybir.dt.bfloat16

B, L1, LQ, LA, H = 32, 512, 32, 32, 128
M = LQ + LA              # 64 tokens per timestep
K = B * M                # 2048 flattened history
NCORES = 8
LC = L1 // NCORES        # 64 l-rows per core
S0 = 40.0                # exp shift (scores are >= 0, max ~50)
T0 = 16.0                # phi centering
NG = 8                   # batch groups of 4 (256 l-cols each)
STACK = 4                # k-chunks stacked per PSUM buffer / exp instruction

_NC_CACHE = None


def _chunks_of_group(g):
    return 2 * g + 2


def _build():
    nc = bacc.Bacc("TRN2", target_bir_lowering=False, debug=False)

    # host-packed inputs (DMA'd piece-wise in consumption order):
    #   inp1 = [Wt (128 cols) | bcol (1 col, f32 bits) | dlgT (d x k)] f32r
    #   inp2 = xdT  (d x (b,l)) for this core's l-slice   f32r-rounded
    #   inp3 = dlg_aug, phi-scaled values + ones column, chunk-tiled bf16
    inp1 = nc.dram_tensor("inp1", [H, 129 + K], F32R,
                          kind="ExternalInput").ap()
    inp2 = nc.dram_tensor("inp2", [H, B * LC], F32R, kind="ExternalInput").ap()
    inp3 = nc.dram_tensor("inp3", [128, 16 * 129], BF16, kind="ExternalInput").ap()

    out = nc.dram_tensor("out", [B, LC, H], F32, kind="ExternalOutput").ap()
    out_flat = out.rearrange("b l d -> (b l) d")  # (2048, 128)

    GROUP_ORDER = [0, 1, 3, 4, 5, 6, 7, 2]
    LAST_G = GROUP_ORDER[-1]

    with tile.TileContext(nc) as tc:
        with tc.tile_pool(name="const", bufs=1) as cpool, \
             tc.tile_pool(name="pt", bufs=4) as ptpool, \
             tc.tile_pool(name="osb", bufs=3) as osbpool, \
             tc.tile_pool(name="ps_big", bufs=2, space="PSUM") as psb, \
             tc.tile_pool(name="ps_out", bufs=2, space="PSUM") as pso:

            negs0 = cpool.tile([128, 1], F32)
            nc.vector.memset(negs0[:], -S0)
            # dummy activation at t~0: forces the 1283ns act-table load
            # (auto-inserted before the first InstActivation) to overlap the
            # DMA prologue instead of gating the first exp.
            warm = cpool.tile([128, 1], F32)
            nc.scalar.activation(warm[:], negs0[:],
                                 mybir.ActivationFunctionType.Exp,
                                 bias=negs0[:], scale=1.0)

            # PE pstate warm-up: keep the tensor engine busy on junk matmuls
            # while the first input DMAs land, so the real prologue matmuls
            # run at the ramped clock.
            wdum = cpool.tile([128, 128], BF16)
            rdum = cpool.tile([128, 512], BF16)
            nc.vector.memset(wdum[:].bitcast(F32), 0)
            nc.vector.memset(rdum[:].bitcast(F32), 0)
            for w in range(4):
                pdum = psb.tile([128, 512], F32, tag="proj", bufs=2,
                                name=f"pdum{w}")
                nc.tensor.matmul(pdum[:], wdum[:], rdum[:],
                                 start=True, stop=True)

            i1_sb = cpool.tile([H, 129 + K], F32R)
            wt_sb = i1_sb[:, 0:128]
            bcol_sb = i1_sb[:, 128:129].bitcast(F32)
            dlgT_sb = i1_sb[:, 129:129 + K]
            xdT_sb = cpool.tile([H, B * LC], F32R)
            i3_sb = cpool.tile([128, 16 * 129], BF16)
            dlga_sb = i3_sb[:]  # (128, 2064)

            # DMAs in consumption order (SP issue + HWDGE drain are FIFO;
            # ~1.2us of fixed per-DMA latency, so the head pieces are small
            # for latency and the tail pieces are big for issue-rate).
            nc.sync.dma_start(i1_sb[:, 0:385], inp1[:, 0:385])        # Wt+y0
            nc.sync.dma_start(xdT_sb[:, 0:512], inp2[:, 0:512])       # x g0,g1
            nc.sync.dma_start(i1_sb[:, 385:641], inp1[:, 385:641])    # y1
            nc.sync.dma_start(xdT_sb[:, 768:1280], inp2[:, 768:1280])  # x g3,g4
            nc.sync.dma_start(i1_sb[:, 641:1153], inp1[:, 641:1153])  # y2,y3
            nc.sync.dma_start(i3_sb[:, 0:1032], inp3[:, 0:1032])      # dlga lo
            nc.sync.dma_start(xdT_sb[:, 1280:2048], inp2[:, 1280:2048])
            nc.sync.dma_start(i1_sb[:, 1153:1665], inp1[:, 1153:1665])  # y4,y5
            nc.sync.dma_start(i1_sb[:, 1665:2177], inp1[:, 1665:2177])  # y6,y7
            nc.sync.dma_start(i3_sb[:, 1032:2064], inp3[:, 1032:2064])  # dlga hi
            nc.sync.dma_start(xdT_sb[:, 512:768], inp2[:, 512:768])   # x g2

            # projections: out[h, col] = relu(sum_d Wt[d, h] * inT[d, col] + b[h])
            # emitted lazily in 256-col pieces; y pieces 0/1 relu on ACT (idle
            # during the prologue), everything else on DVE.
            yproj = cpool.tile([H, K], F32R)
            xproj = cpool.tile([H, B * LC], F32R)

            next_y = [0]
            x_done = set()

            def proj_ps(name):
                return psb.tile([128, 512], F32, tag="proj", bufs=2,
                                name=f"psproj_{name}")

            def emit_relu(dst_slice, ps_slice, on_act):
                if on_act:
                    nc.scalar.activation(dst_slice, ps_slice,
                                         mybir.ActivationFunctionType.Relu,
                                         bias=bcol_sb, scale=1.0)
                else:
                    nc.vector.tensor_scalar(dst_slice, ps_slice, bcol_sb, 0.0,
                                            op0=mybir.AluOpType.add,
                                            op1=mybir.AluOpType.max)

            def need_y(upto, ps_slack=None):
                while next_y[0] * 256 < upto:
                    k = next_y[0]
                    lo = k * 256
                    ps = ps_slack if ps_slack is not None else \
                        proj_ps(f"y{k}")[:, 0:256]
                    ps_slack = None
                    nc.tensor.matmul(ps, wt_sb, dlgT_sb[:, lo:lo + 256],
                                     start=True, stop=True)
                    emit_relu(yproj[:, lo:lo + 256], ps, k < 2)
                    next_y[0] += 1

            def need_x(g, ps_slack=None):
                if g in x_done:
                    return
                x_done.add(g)
                lo = g * 256
                ps = ps_slack if ps_slack is not None else \
                    proj_ps(f"x{g}")[:, 0:256]
                nc.tensor.matmul(ps, wt_sb, xdT_sb[:, lo:lo + 256],
                                 start=True, stop=True)
                emit_relu(xproj[:, lo:lo + 256], ps, False)

            # flat stack list; each stack is (g, [chunk indices by position]).
            # The final stack of a group leads with the LAST chunk: its first
            # 128 pt cols are causally dead (psA skipped), so the exp window
            # starts at col 128 — saves 128 ACT columns per group.
            def stack_sizes(nchunks):
                sizes = []
                rem = nchunks
                while rem > 0:
                    sizes.append(min(STACK, rem))
                    rem -= min(STACK, rem)
                return sizes

            # A group's short (<=2 chunk) closing stack cannot hide the next
            # group's scores latency behind its 505ns exp, so for mid-train
            # groups it is processed SECOND instead of last — the group then
            # closes on a full-size exp.  psA/psB accumulation flags are
            # assigned by processing position (start on first, stop on last),
            # so any chunk processing order is valid.
            group_stacks = {}
            for g in GROUP_ORDER:
                nchunks = _chunks_of_group(g)
                s0 = 0
                stacks = []
                sizes = stack_sizes(nchunks)
                for si, ns in enumerate(sizes):
                    if si == len(sizes) - 1:
                        stacks.append([nchunks - 1] + list(range(s0, nchunks - 1)))
                    else:
                        stacks.append(list(range(s0, s0 + ns)))
                    s0 += ns
                group_stacks[g] = stacks

            flat = []
            first_psA = {}
            last_psA = {}
            last_psB = {}
            for g in GROUP_ORDER:
                nchunks = _chunks_of_group(g)
                seq = [c for st in group_stacks[g] for c in sorted(st)]
                psA_seq = [c for c in seq if c < nchunks - 1]
                first_psA[g] = psA_seq[0]
                last_psA[g] = psA_seq[-1]
                last_psB[g] = seq[-1]
                for si, st in enumerate(group_stacks[g]):
                    flat.append((g, st, si == len(group_stacks[g]) - 1))

            state = {}   # g -> psAB
            tiles = {}   # i -> (ps, pt)

            def emit_scores(i):
                g, chunks, _final = flat[i]
                xg = xproj[:, g * 256:(g + 1) * 256]
                if i in tiles:
                    ps, pt = tiles[i]
                else:
                    if i > 0 and len(chunks) <= 2:
                        # short final stacks borrow the proj ring's 512-col
                        # slots: their 505ns exp cannot hide the next group's
                        # scores latency, so keeping them out of the main
                        # scps rotation lets the next scores start two exps
                        # earlier.
                        ps = proj_ps(f"sc_s{i}")[:, 0:512]
                    else:
                        ps = psb.tile([128, STACK * 256], F32, tag="scps")
                    pt = ptpool.tile([128, STACK * 256], BF16, tag="pt")
                    tiles[i] = (ps, pt)
                for k, c in enumerate(chunks):
                    nc.tensor.matmul(ps[:, k * 256:(k + 1) * 256],
                                     yproj[:, c * 128:(c + 1) * 128], xg,
                                     start=True, stop=True)

            def emit_process(i):
                g, chunks, final = flat[i]
                ns = len(chunks)
                nchunks = _chunks_of_group(g)
                ps, pt = tiles.pop(i)
                # the stack leading with the group's last chunk has dead cols
                # 0:128 (psA skipped there): start the exp window at col 128.
                off = 128 if chunks[0] == nchunks - 1 else 0
                nc.scalar.activation(pt[:, off:ns * 256], ps[:, off:ns * 256],
                                     mybir.ActivationFunctionType.Exp,
                                     bias=negs0[:], scale=1.0)
                for k, c in enumerate(chunks):
                    blk = pt[:, k * 256:(k + 1) * 256]
                    if c == nchunks - 2:
                        nc.vector.memset(blk[:, 0:64], 0)
                        nc.vector.memset(blk[64:128, 64:128], 0)
                    elif c == nchunks - 1:
                        # cols 0:128 are never read (psA matmul skipped)
                        nc.gpsimd.memset(blk[:, 128:192], 0)
                        nc.gpsimd.memset(blk[64:128, 192:256], 0)
                if min(chunks) == 0:
                    # psA/psB share one PSUM bank: psA's start=True clears the
                    # bank's has_written bits, so psB's first matmul must use
                    # start=False (overwrites the still-clear region).
                    psAB = pso.tile([128, 260], F32, tag="psout", name=f"ps{g}")
                    state[g] = psAB
                psAB = state[g]
                psA = psAB[:, 0:129]
                psB = psAB[:, 130:259]
                # out-MMs in ascending chunk order within the stack; start/
                # stop flags follow the group's chunk PROCESSING order (psA's
                # start=True must be the bank's first write).
                for c in sorted(chunks):
                    k = chunks.index(c)
                    dchunk = dlga_sb[:, c * 129:(c + 1) * 129]
                    blk = pt[:, k * 256:(k + 1) * 256]
                    if c < nchunks - 1:
                        # last chunk's cols 0:128 are causally all-zero: skip
                        nc.tensor.matmul(psA, blk[:, 0:128], dchunk,
                                         start=(c == first_psA[g]),
                                         stop=(c == last_psA[g]))
                    nc.tensor.matmul(psB, blk[:, 128:256], dchunk,
                                     start=False, stop=(c == last_psB[g]))
                if final:
                    emit_normalize(g)

            def emit_normalize(g):
                psAB = state.pop(g)
                psA = psAB[:, 0:129]
                psB = psAB[:, 130:259]
                osb = osbpool.tile([128, 256], F32, tag="osb")
                # both recips first, then both multiplies: saves one DVE
                # serialization hop on the critical tail.
                recips = []
                for half, pshalf in ((0, psA), (1, psB)):
                    recip = osbpool.tile([128, 1], F32, tag="recip")
                    recips.append(recip)
                    if g == 0 and half == 0:
                        nc.vector.reciprocal(recip[64:128, :],
                                             pshalf[64:128, 128:129])
                    else:
                        nc.vector.reciprocal(recip[:], pshalf[:, 128:129])
                for half, pshalf in ((0, psA), (1, psB)):
                    dst = osb[:, half * 128:(half + 1) * 128]
                    recip = recips[half]
                    if g == 0 and half == 0:
                        # batch 0: output is defined as zero (denominator is 0)
                        nc.vector.memset(dst[0:64, :], 0)
                        nc.vector.tensor_scalar_mul(dst[64:128, :],
                                                    pshalf[64:128, 0:128],
                                                    recip[64:128, :])
                    else:
                        nc.vector.tensor_scalar_mul(dst[:], pshalf[:, 0:128],
                                                    recip[:])
                dsl = out_flat[256 * g:256 * (g + 1)].rearrange(
                    "(h p) d -> p h d", h=2)
                osb_src = osb[:].rearrange("p (h d) -> p h d", h=2)
                # Output DMA routing: mid-train groups go through Pool's
                # software DGE (their data-waits would block the SP queue head
                # and delay the final DMA).  The second-to-last group's DMA is
                # deferred to the post-loop ACT queue so its transfer doesn't
                # occupy the DMA lane right when the final DMA (on the
                # then-empty SP queue) needs it.
                if g == LAST_G:
                    nc.sync.dma_start(dsl, osb_src)
                elif g == GROUP_ORDER[-2]:
                    deferred_dma.append((dsl, osb_src))
                else:
                    nc.gpsimd.dma_start(dsl, osb_src)

            # software pipeline: scores(i) ahead of process(i-LOOKAHEAD).
            # process(0) is emitted right after scores(0) so exp(0) sits ahead
            # of the later y-relus in the ACT queue.
            LOOKAHEAD = 2
            done_proc = set()
            deferred_dma = []

            def proc(j):
                if 0 <= j < len(flat) and j not in done_proc:
                    done_proc.add(j)
                    emit_process(j)

            for i in range(len(flat) + LOOKAHEAD):
                if i < len(flat):
                    if i == 0:
                        # stack 0 uses only 512 of its 1024 PSUM cols; put the
                        # two prologue projection pieces (y0, x-g0) in the
                        # slack so they don't serialize on the proj ring.
                        ps0 = psb.tile([128, STACK * 256], F32, tag="scps",
                                       name="ps_s0")
                        pt0 = ptpool.tile([128, STACK * 256], BF16, tag="pt",
                                          name="pt_s0")
                        tiles[0] = (ps0, pt0)
                        need_y(256, ps_slack=ps0[:, 512:768])
                        need_x(GROUP_ORDER[0], ps_slack=ps0[:, 768:1024])
                    # pre-emit projection pieces ~2 stacks ahead: the relu for
                    # the NEXT group must enter the DVE queue before this
                    # group's normalize, or the next group's scores stall.
                    for j in range(i, min(i + 3, len(flat))):
                        gj, chunksj, _ = flat[j]
                        need_y(128 * (max(chunksj) + 1))
                        need_x(gj)
                    emit_scores(i)
                    if i == 0:
                        proc(0)
                proc(i - LOOKAHEAD)

            # second-to-last group's output DMA, issued from the ACT queue
            # after the last exp decode (ACT is idle by then).
            for dsl, osb_src in deferred_dma:
                nc.scalar.dma_start(dsl, osb_src)

    nc.compile()
    return nc


def _get_nc():
    global _NC_CACHE
    if _NC_CACHE is None:
        _NC_CACHE = _build()
    return _NC_CACHE


def _round_f32r(a):
    u = np.ascontiguousarray(a, dtype=np.float32).view(np.uint32)
    r = ((u.astype(np.uint64) + 0x800) & 0xFFFFF000).astype(np.uint32)
    return r.view(np.float32)


LAST_RESULTS = None  # BassKernelResults of the most recent run (for test harness)


def kernel(xd_emb, xq_emb, xa_emb, W, b, recency_weight, xq_mask, xa_mask,
           _trace=False):
    xd_emb = np.asarray(xd_emb, np.float32)
    xq_emb = np.asarray(xq_emb, np.float32)
    xa_emb = np.asarray(xa_emb, np.float32)
    W = np.asarray(W, np.float32)
    b = np.asarray(b, np.float32)
    rw0 = float(np.asarray(recency_weight).reshape(-1)[0])
    pad = np.concatenate([np.asarray(xq_mask), np.asarray(xa_mask)], axis=1).reshape(K)

    dlg = np.concatenate([xq_emb, xa_emb], axis=1).reshape(K, H)
    ts = (np.arange(K) // M).astype(np.float64)
    phi = np.exp(-rw0 * (ts - T0))
    dlg_aug = np.concatenate([dlg.astype(np.float64), np.ones((K, 1))], axis=1)
    dlg_aug *= phi[:, None]
    dlg_aug[pad] = 0.0
    dlga_bf = dlg_aug.astype(ml_dtypes.bfloat16)
    dlga_packed = np.ascontiguousarray(
        dlga_bf.reshape(16, 128, 129).transpose(1, 0, 2).reshape(128, 16 * 129))

    inp1 = np.empty((H, 129 + K), np.float32)
    inp1[:, 0:128] = _round_f32r(W.T)
    inp1[:, 128] = b
    inp1[:, 129:] = _round_f32r(dlg.T)
    inp3 = dlga_packed  # (128, 2064) bf16

    xdT = xd_emb.transpose(2, 0, 1)  # (H, B, L1)
    in_maps = []
    for c in range(NCORES):
        xdT_c = xdT[:, :, c * LC:(c + 1) * LC].reshape(H, B * LC)
        in_maps.append({
            "inp1": inp1,
            "inp2": _round_f32r(xdT_c),
            "inp3": inp3,
        })

    nc = _get_nc()
    try:
        res = run_bass_kernel_spmd(nc, in_maps, list(range(NCORES)),
                                   trace=_trace)
    except ModuleNotFoundError:
        # The axon NTFF-profile hook is absent in this container; if an
        # ambient BASS_TRACE forced the trace path, retry without it.
        os.environ["BASS_NEVER_TRACE"] = "1"
        res = run_bass_kernel_spmd(nc, in_maps, list(range(NCORES)))
    global LAST_RESULTS
    LAST_RESULTS = res
    parts = [res.results[c]["out"] for c in range(NCORES)]
    full = np.concatenate(parts, axis=1)  # (32, 512, 128)
    full[0] = 0.0
    return np.ascontiguousarray(full, dtype=np.float32)
